# revision 8
# baseline (speedup 1.0000x reference)
"""Single-head causal attention on 8 TRN2 NeuronCores, batch-parallel.

Problem: x[8,2048,1024] f32, Wq/Wk/Wv[1024,64] f32
  q,k,v = x@W*  ;  scores = q k^T / sqrt(1024), causal  ;  out = softmax(scores) @ v

Sharding: batch dim across 8 cores (1 batch element per core, no collectives).

Per-core dataflow (all matmuls fp32r = tf32-class, 1 cyc/row at N>=256):
  A) xT: PE-transpose x [2048,1024] -> xT [c=128part x 8ct, t=2048]
  B) proj: lhsT=[Wq|Wv] -> qvT psum [128,512] (rows 0:64 qT, 64:128 vT);
     kT separate (M=64).  vT -> PE-transpose -> v_aug [128s,16,65] with ones col.
  C) per t-chunk (512): st_j [s=128, t=512] = kT_j^T q ; exp(st/32) -> wst (f32r);
     causal mask on diagonal tiles (memset + 0/1 triangle multiply);
     psum_o [65,512] += v_aug_j^T wst_j  (row 64 = softmax denominator);
     PE-transpose psum_o -> [128t, 65], out = cols0:64 * recip(col 64) -> HBM.
"""

import numpy as np

import concourse.bacc as bacc
import concourse.mybir as mybir
import concourse.tile as tile
from concourse.bass_utils import run_bass_kernel_spmd

F32 = mybir.dt.float32
F32R = mybir.dt.float32r

B, T, C, H = 8, 2048, 1024, 64
NCT = C // 128          # 8 c-tiles
NTT = T // 128          # 16 t/s-tiles
NCH = T // 512          # 4 t-chunks
SCALE = float(C ** -0.5)

_CACHE = {}


def build():
    nc = bacc.Bacc(name="head_attn")
    x_d = nc.dram_tensor("x", [T, C], F32, kind="ExternalInput")
    wq_d = nc.dram_tensor("Wq", [C, H], F32, kind="ExternalInput")
    wk_d = nc.dram_tensor("Wk", [C, H], F32, kind="ExternalInput")
    wv_d = nc.dram_tensor("Wv", [C, H], F32, kind="ExternalInput")
    id_d = nc.dram_tensor("ident", [128, 128], F32, kind="ExternalInput")
    ih_d = nc.dram_tensor("identhi", [128, 64], F32, kind="ExternalInput")
    tri_d = nc.dram_tensor("tri", [128, 128], F32, kind="ExternalInput")
    msk_d = nc.dram_tensor("masks", [128, 4, 512], F32, kind="ExternalInput")
    one_d = nc.dram_tensor("ones16", [128, 16, 2], F32, kind="ExternalInput")
    out_d = nc.dram_tensor("out", [T, H], F32, kind="ExternalOutput")

    with tile.TileContext(nc) as tc:
        with (
            tc.tile_pool(name="singles", bufs=1) as singles,
            tc.tile_pool(name="stage", bufs=4) as stage,
            tc.tile_pool(name="work", bufs=8) as work,
            tc.tile_pool(name="outp", bufs=4) as outp,
            tc.tile_pool(name="pbig", bufs=4, space="PSUM") as pbig,
            tc.tile_pool(name="pacc", bufs=2, space="PSUM") as pacc,
            tc.tile_pool(name="psmall", bufs=2, space="PSUM") as psmall,
        ):
            # ---- constants / weights
            ident = singles.tile([128, 128], F32R)
            identhi = singles.tile([128, 64], F32R)
            tri = singles.tile([128, 128], F32R)
            nc.sync.dma_start(ident, id_d[:, :].bitcast(F32R))
            nc.sync.dma_start(identhi, ih_d[:, :].bitcast(F32R))
            nc.sync.dma_start(tri, tri_d[:, :].bitcast(F32R))
            masks = singles.tile([128, 4, 512], F32R)
            nc.sync.dma_start(masks, msk_d[:, :, :].bitcast(F32R))

            wqv = singles.tile([128, NCT, 128], F32R)   # [c | ct | (q h, v h)]
            wk = singles.tile([128, NCT, H], F32R)
            nc.sync.dma_start(
                wqv[:, :, 0:H], wq_d.rearrange("(t c) h -> c t h", c=128).bitcast(F32R))
            nc.sync.dma_start(
                wqv[:, :, H:128], wv_d.rearrange("(t c) h -> c t h", c=128).bitcast(F32R))
            nc.sync.dma_start(
                wk[:, :, :], wk_d.rearrange("(t c) h -> c t h", c=128).bitcast(F32R))

            xT = singles.tile([128, NCT, T], F32R)      # [c, ct, t]

            # ---- A) transpose x into xT
            for tt in range(NTT):
                xs = stage.tile([128, C], F32R, tag="xs")
                nc.sync.dma_start(xs, x_d[tt * 128:(tt + 1) * 128, :].bitcast(F32R))
                for cg in range(2):                     # 2 groups of 4 c-tiles
                    pt = pbig.tile([128, 512], F32R, tag="big")
                    for k in range(4):
                        ct = cg * 4 + k
                        nc.tensor.transpose(
                            pt[:, k * 128:(k + 1) * 128],
                            xs[:, ct * 128:(ct + 1) * 128], ident)
                    dst = xT[:, cg * 4:(cg + 1) * 4, tt * 128:(tt + 1) * 128]
                    if (2 * tt + cg) % 8 < 5:
                        nc.scalar.copy(dst, pt.rearrange("p (a b) -> p a b", a=4))
                    else:
                        nc.vector.tensor_copy(dst, pt.rearrange("p (a b) -> p a b", a=4))

            # ---- B) projections
            qvT = singles.tile([128, T], F32R)          # rows 0:64 qT, 64:128 vT
            kT = singles.tile([64, T], F32R)
            for i in range(NCH):
                pq = pbig.tile([128, 512], F32, tag="big")
                for ct in range(NCT):
                    nc.tensor.matmul(pq, wqv[:, ct, :], xT[:, ct, i * 512:(i + 1) * 512],
                                     start=(ct == 0), stop=(ct == NCT - 1))
                nc.vector.tensor_copy(qvT[:, i * 512:(i + 1) * 512].bitcast(F32R), pq)
                pk = pbig.tile([64, 512], F32, tag="big")
                for ct in range(NCT):
                    nc.tensor.matmul(pk, wk[:, ct, :], xT[:, ct, i * 512:(i + 1) * 512],
                                     start=(ct == 0), stop=(ct == NCT - 1))
                nc.vector.tensor_copy(kT[:, i * 512:(i + 1) * 512].bitcast(F32R), pk)

            # v_aug [s=128, 16, 66] with two ones columns (66 keeps the
            # fp32r output transpose even-sized and partition-base aligned)
            v_aug = singles.tile([128, NTT, 66], F32R)
            nc.sync.dma_start(v_aug[:, :, 64:66], one_d[:, :, :].bitcast(F32R))
            for s in range(NTT):
                pv = psmall.tile([128, 64], F32R, tag="small")
                nc.tensor.transpose(
                    pv, qvT[64:128, s * 128:(s + 1) * 128], identhi[64:128, :])
                nc.vector.tensor_copy(v_aug[:, s, 0:64], pv)

            # ---- C) attention
            for i in range(NCH):
                po = pacc.tile([66, 512], F32)
                nj = 4 * i + 4
                for j in range(nj):
                    pst = pbig.tile([128, 512], F32, tag="big")
                    nc.tensor.matmul(pst, kT[:, j * 128:(j + 1) * 128],
                                     qvT[0:64, i * 512:(i + 1) * 512],
                                     start=True, stop=True)
                    wst = work.tile([128, 512], F32R, tag="wst")
                    k = j - 4 * i
                    d = 128 * k if k > 0 else 0
                    nc.scalar.activation(wst[:, d:], pst[:, d:],
                                         mybir.ActivationFunctionType.Exp, scale=SCALE)
                    if k >= 0:                           # diagonal: mask the triangle
                        nc.vector.tensor_mul(wst[:, d:d + 128], wst[:, d:d + 128], tri)
                    nc.tensor.matmul(po[:, d:], v_aug[:, j, :], wst[:, d:],
                                     start=(j == 0), stop=(j == nj - 1))

                oT = outp.tile([66, 512], F32R, tag="oT")
                nc.scalar.copy(oT, po)
                for b in range(4):
                    pn = psmall.tile([128, 66], F32R, tag="small")
                    nc.tensor.transpose(pn, oT[:, b * 128:(b + 1) * 128],
                                        ident[0:66, 0:66])
                    rec = outp.tile([128, 1], F32, tag="rec")
                    nc.vector.reciprocal(rec, pn[:, 64:65])
                    ob = outp.tile([128, 64], F32, tag="ob")
                    nc.vector.tensor_scalar_mul(ob, pn[:, 0:64], rec)
                    nc.sync.dma_start(
                        out_d[i * 512 + b * 128: i * 512 + (b + 1) * 128, :], ob)

    nc.compile()
    return nc


def _consts():
    ident = np.eye(128, dtype=np.float32)
    identhi = np.zeros((128, 64), dtype=np.float32)
    identhi[64:128, :] = np.eye(64, dtype=np.float32)
    # tri[p, v] = 1 where v >= p  (valid, upper incl diag in [s, u] coords)
    tri = np.triu(np.ones((128, 128), dtype=np.float32))
    return ident, identhi, tri


def kernel(x, Wq, Wk, Wv, trace=False):
    x = np.ascontiguousarray(np.asarray(x, dtype=np.float32))
    Wq = np.ascontiguousarray(np.asarray(Wq, dtype=np.float32))
    Wk = np.ascontiguousarray(np.asarray(Wk, dtype=np.float32))
    Wv = np.ascontiguousarray(np.asarray(Wv, dtype=np.float32))

    if "nc" not in _CACHE:
        _CACHE["nc"] = build()
    nc = _CACHE["nc"]

    ident, identhi, tri = _consts()
    p = np.arange(128, dtype=np.float32)[:, None]
    u = np.arange(512, dtype=np.float32)[None, :]
    masks = np.stack([(u >= p + 128 * k).astype(np.float32) for k in range(4)], axis=1)
    ones16 = np.ones((128, 16, 2), dtype=np.float32)
    in_maps = [
        {"x": x[b], "Wq": Wq, "Wk": Wk, "Wv": Wv,
         "ident": ident, "identhi": identhi, "tri": tri,
         "masks": masks, "ones16": ones16}
        for b in range(B)
    ]
    try:
        res = run_bass_kernel_spmd(nc, in_maps, core_ids=list(range(B)), trace=trace)
    except ModuleNotFoundError:
        res = run_bass_kernel_spmd(nc, in_maps, core_ids=list(range(B)))
    out = np.stack([r["out"] for r in res.results], axis=0)
    kernel.last_exec_time_ns = res.exec_time_ns
    kernel.last_results = res
    return out
